# revision 53
# baseline (speedup 1.0000x reference)
"""Trainium2 Bass kernel for nn_AggrHGraphConvWindow (hetero GraphConv + 2-layer LSTM).

Sharding: data-parallel over the 2000-row batch axis across 8 NeuronCores.
Rows are padded/permuted to 2048 = 8 x 256 so every core runs an identical
program on blocks [node 13 | pod 188 | svc 50 | pad 5].  Source features are
replicated; each core holds the adjacency slice for its own destination rows,
so there are no collectives at all.

Device program per core (all matmuls bf16, PSUM fp32):
  conv stage 1:  agg_t^T [64,256] = sum_k xsrc_k[:, t]^T @ A^T_k
  conv stage 2:  x_t^T [128,256] = Wconv_ty^T @ [agg_t^T; ones-row]
                 (per-row-type weights; conv bias rides the ones row)
                 x_t = ACT Prelu(psum, alpha=.01) -> SBUF bf16  (leaky relu;
                 Prelu lives in the same ACT table set as sigmoid/tanh)
  LSTM (2 layers), gate blocks host-reordered to [i, g, f, o]:
      f,o matmuls emitted first (their sigmoid path feeds the cell update),
      then i,g; bias added on DVE from PSUM (bf16 out), sigmoid/tanh on ACT,
      cell update on DVE, tanh(c) + h-mul split in halves.
      The t-wavefront runs L1 TWO steps behind L0 so every L1 matmul operand
      is ready when its round starts and the L0 recurrence chain hides under
      L1's matmul work; conv timesteps are woven into the rounds to fill the
      remaining PE gaps.
      h2 DMA'd to DRAM [T,128,512] bf16; host un-permutes to [B,T,H] fp32.
"""

import numpy as np
import ml_dtypes

N_NODE, N_POD, N_SVC = 100, 1500, 400
T, F, IN, H = 32, 64, 128, 256
B = 256                       # rows per core (incl. padding)
NODE_OFF, POD_OFF, SVC_OFF = 0, 13, 201   # block offsets within a core's 256 rows
S = 2048                      # padded src rows: pod 0:1500 | node 1500:1600 | svc 1600:2000 | pad
NK_SRC = S // 128             # 16 k-tiles for conv stage 1
NCORES = 8
BF16 = ml_dtypes.bfloat16
GATE_PERM = [0, 2, 1, 3]      # new gate block order [i, g, f, o] (old i,f,g,o)
SIM_SAFE = False              # True: substitute Relu for Lrelu (CoreSim lacks Lrelu)

_BUILT = None   # cached compiled Bass program


def _build():
    import concourse.bass as bass  # noqa: F401
    import concourse.mybir as mybir
    import concourse.tile as tile
    from concourse import bacc
    from contextlib import ExitStack

    f32 = mybir.dt.float32
    bf16 = mybir.dt.bfloat16
    AF = mybir.ActivationFunctionType

    nc = bacc.Bacc("TRN2", target_bir_lowering=False, debug=False,
                   num_devices=NCORES)

    xsrc_d = nc.dram_tensor("xsrc", [S, T * F], bf16, kind="ExternalInput")
    at_d = nc.dram_tensor("at", [128, NK_SRC * B], bf16, kind="ExternalInput")
    wconv_d = nc.dram_tensor("wconv", [65, T * 384], bf16, kind="ExternalInput")
    wih0_d = nc.dram_tensor("wih0", [128, 1024], bf16, kind="ExternalInput")
    whh0_d = nc.dram_tensor("whh0", [128, 2048], bf16, kind="ExternalInput")
    wih1_d = nc.dram_tensor("wih1", [128, 2048], bf16, kind="ExternalInput")
    whh1_d = nc.dram_tensor("whh1", [128, 2048], bf16, kind="ExternalInput")
    b0_d = nc.dram_tensor("b0rep", [128, 2048], bf16, kind="ExternalInput")
    b1_d = nc.dram_tensor("b1rep", [128, 2048], bf16, kind="ExternalInput")
    b0c_d = nc.dram_tensor("b0c", [128, 8], f32, kind="ExternalInput")
    b1c_d = nc.dram_tensor("b1c", [128, 8], f32, kind="ExternalInput")
    brow_d = nc.dram_tensor("brow", [1, 2048], bf16, kind="ExternalInput")
    out_d = nc.dram_tensor("out", [T, 128, 512], bf16, kind="ExternalOutput")

    with tile.TileContext(nc) as tc, ExitStack() as ctx:
        const = ctx.enter_context(tc.tile_pool(name="const", bufs=1))
        psp = ctx.enter_context(tc.tile_pool(name="psp", bufs=4, space="PSUM"))
        gp = ctx.enter_context(tc.tile_pool(name="gp", bufs=4))
        wk = ctx.enter_context(tc.tile_pool(name="wk", bufs=2))
        aggp = ctx.enter_context(tc.tile_pool(name="aggp", bufs=4))
        wcp = ctx.enter_context(tc.tile_pool(name="wcp", bufs=3))
        hop = ctx.enter_context(tc.tile_pool(name="hop", bufs=2))
        cp = ctx.enter_context(tc.tile_pool(name="cp", bufs=4))

        # ---- persistent loads ----
        # small operands first so conv matmuls can start as xsrc k-tiles land
        at = const.tile([128, NK_SRC * B], bf16)
        nc.sync.dma_start(out=at[:], in_=at_d.ap())
        wih0 = const.tile([128, 1024], bf16)
        nc.sync.dma_start(out=wih0[:], in_=wih0_d.ap())
        xsrc = const.tile([128, NK_SRC * 2048], bf16)
        for k in range(NK_SRC):
            nc.sync.dma_start(out=xsrc[:, k * 2048:(k + 1) * 2048],
                              in_=xsrc_d.ap()[k * 128:(k + 1) * 128, :])
        whh0 = const.tile([128, 2048], bf16)
        nc.sync.dma_start(out=whh0[:], in_=whh0_d.ap())
        wih1 = const.tile([128, 2048], bf16)
        nc.sync.dma_start(out=wih1[:], in_=wih1_d.ap())
        whh1 = const.tile([128, 2048], bf16)
        nc.sync.dma_start(out=whh1[:], in_=whh1_d.ap())
        b0rep = const.tile([128, 2048], bf16)
        nc.sync.dma_start(out=b0rep[:], in_=b0_d.ap())
        b1rep = const.tile([128, 2048], bf16)
        nc.sync.dma_start(out=b1rep[:], in_=b1_d.ap())
        b0c = const.tile([128, 8], f32)
        nc.sync.dma_start(out=b0c[:], in_=b0c_d.ap())
        b1c = const.tile([128, 8], f32)
        nc.sync.dma_start(out=b1c[:], in_=b1c_d.ap())
        brow = const.tile([1, 2048], bf16)
        nc.sync.dma_start(out=brow[:], in_=brow_d.ap())
        ones = const.tile([1, B], bf16)
        nc.vector.memset(ones[:], 1.0)

        x_sb = const.tile([128, T * B], bf16)        # conv output (LSTM L0 input)
        h1f = const.tile([128, T * 512], bf16)       # L0 hidden states (L1 input)

        # ---- conv ----
        blocks = [(NODE_OFF, POD_OFF - NODE_OFF),
                  (POD_OFF, SVC_OFF - POD_OFF),
                  (SVC_OFF, B - SVC_OFF)]
        def conv_t(t):
            agg = psp.tile([64, B], f32, tag="psA", bufs=4)
            for k in range(NK_SRC):
                nc.tensor.matmul(
                    agg[:, :],
                    xsrc[:, k * 2048 + t * 64: k * 2048 + (t + 1) * 64],
                    at[:, k * B:(k + 1) * B],
                    start=(k == 0), stop=(k == NK_SRC - 1))
            aggT = aggp.tile([65, B], bf16, tag="aggT")
            nc.vector.tensor_copy(aggT[0:64, :], agg[:, :])
            nc.gpsimd.memset(aggT[64:65, :], 1.0)
            wct = wcp.tile([65, 384], bf16, tag="wct")
            nc.sync.dma_start(out=wct[:],
                              in_=wconv_d.ap()[:, t * 384:(t + 1) * 384])
            xps = psp.tile([128, B], f32, tag="psA", bufs=4)
            for ty, (off, wid) in enumerate(blocks):
                nc.tensor.matmul(xps[:, off:off + wid],
                                 wct[:, ty * 128:(ty + 1) * 128],
                                 aggT[:, off:off + wid],
                                 start=True, stop=True)
            nc.scalar.activation(x_sb[:, t * B:(t + 1) * B], xps[:, :],
                                 AF.Relu if SIM_SAFE else AF.Prelu,
                                 alpha=0.01)

        # ---- LSTM ----
        state = {0: {"c": None}, 1: {"c": None, "h": None}}

        def lstm_step(layer, t):
            # kparts ordered so ready-early operands issue first
            if layer == 0:
                kparts = [(wih0, 1, x_sb[:, t * B:(t + 1) * B])]
                if t > 0:
                    kparts.append((whh0, 2, h1f[:, (t - 1) * 512: t * 512]))
                brep, bc, boff = b0rep, b0c, 0
            else:
                kparts = [(whh1, 2, state[1]["h"][:])] if t > 0 else []
                kparts.append((wih1, 2, h1f[:, t * 512:(t + 1) * 512]))
                brep, bc, boff = b1rep, b1c, 1024
            nmm = sum(nk for (_, nk, _) in kparts)

            def emit_mms(ps, pcol, cth):
                i_mm = 0
                for (w, nk, rhs) in kparts:
                    for kk in range(nk):
                        nc.tensor.matmul(
                            ps[:, pcol * 256:(pcol + 1) * 256],
                            w[:, kk * 1024 + cth * 128: kk * 1024 + (cth + 1) * 128],
                            rhs[:, kk * B:(kk + 1) * B],
                            start=(i_mm == 0), stop=(i_mm == nmm - 1))
                        i_mm += 1

            # gate order [i, g, f, o]: i,g chain-critical -> per-chunk biased
            # ACT straight from PSUM; f,o -> wide DVE bias-add + sigmoid.
            # f,o matmuls first: their sigmoid path (p2 = f*c) must be ready
            # by the time the i,g chain reaches the cell update
            psB = psp.tile([128, 1024], f32, tag="psB", bufs=2)
            for c in range(4):
                emit_mms(psB, c, 4 + c)        # chunks 4-7 = gates f,o
            psi = psp.tile([128, 512], f32, tag="psA", bufs=4)
            for c in range(2):
                emit_mms(psi, c, c)            # chunks 0,1 = gate i
            psg = psp.tile([128, 512], f32, tag="psA", bufs=4)
            for c in range(2):
                emit_mms(psg, c, 2 + c)        # chunks 2,3 = gate g
            zB = gp.tile([128, 1024], bf16, tag="zB")
            nc.vector.tensor_add(zB[:], psB[:], brep[:, 1024:2048])
            gB = gp.tile([128, 1024], bf16, tag="gB")
            nc.scalar.activation(gB[:], zB[:], AF.Sigmoid)
            gf, go = gB[:, 0:512], gB[:, 512:1024]
            zi = wk.tile([128, 512], bf16, tag="zi")
            nc.vector.tensor_add(zi[:], psi[:], brep[:, 0:512])
            gi = gp.tile([128, 512], bf16, tag="gi")
            nc.scalar.activation(gi[:], zi[:], AF.Sigmoid)
            zg = wk.tile([128, 512], bf16, tag="zg")
            nc.vector.tensor_add(zg[:], psg[:], brep[:, 512:1024])
            gg = gp.tile([128, 512], bf16, tag="gg")
            nc.scalar.activation(gg[:], zg[:], AF.Tanh)

            p1 = wk.tile([128, 512], bf16, tag="p1")
            nc.vector.tensor_mul(p1[:], gi[:], gg[:])
            cn = cp.tile([128, 512], bf16, tag=f"c{layer}")
            if t == 0:
                nc.vector.tensor_copy(cn[:], p1[:])
            else:
                p2 = wk.tile([128, 512], bf16, tag="p2")
                nc.vector.tensor_mul(p2[:], gf, state[layer]["c"][:])
                nc.vector.tensor_add(cn[:], p1[:], p2[:])
            state[layer]["c"] = cn
            thc = wk.tile([128, 512], bf16, tag="thc")
            if layer == 0:
                hout = h1f[:, t * 512:(t + 1) * 512]
            else:
                hout = hop.tile([128, 512], bf16, tag="h2o")
                state[1]["h"] = hout
            # halves: downstream matmuls consume h k-tile by k-tile
            for hh in range(2):
                sl = slice(hh * 256, (hh + 1) * 256)
                nc.scalar.activation(thc[:, sl], cn[:, sl], AF.Tanh)
                nc.vector.tensor_mul(hout[:, sl],
                                     gB[:, 512 + hh * 256: 512 + (hh + 1) * 256],
                                     thc[:, sl])
            if layer == 1:
                nc.sync.dma_start(out=out_d.ap()[t], in_=hout[:])

        # weave conv pairs into the LSTM wavefront; L1 lags L0 by TWO
        # steps so every L1 matmul input is ready before the round starts
        # and the L0 recurrence chain hides under L1's matmul work
        conv_t(0)
        conv_t(1)
        conv_t(2)
        lstm_step(0, 0)
        lstm_step(0, 1)
        for t in range(2, T):
            lstm_step(1, t - 2)
            lstm_step(0, t)
            if t + 1 < T:
                conv_t(t + 1)
        lstm_step(1, T - 2)
        lstm_step(1, T - 1)

    nc.compile()
    return nc


def _get_built():
    global _BUILT
    if _BUILT is None:
        _BUILT = _build()
    return _BUILT


def _core_row_ids(c):
    """Per-core (node_ids, pod_ids, svc_ids) real row id arrays."""
    n0, n1 = 13 * c, min(13 * (c + 1), N_NODE)
    p0, p1 = 188 * c, min(188 * (c + 1), N_POD)
    s0, s1 = 50 * c, min(50 * (c + 1), N_SVC)
    return np.arange(n0, n1), np.arange(p0, p1), np.arange(s0, s1)


def _gate_reorder(Wm):
    Wm = np.asarray(Wm, np.float32)
    return np.concatenate([Wm[g * 256:(g + 1) * 256] for g in GATE_PERM], axis=0)


def _prep_inputs(node_feat, pod_feat, svc_feat, W_svc, b_svc, W_in, b_in,
                 W_ni, b_ni, W_ih0, W_hh0, b_ih0, b_hh0, W_ih1, W_hh1,
                 b_ih1, b_hh1, svc_src, svc_dst, in_src, in_dst,
                 ni_src, ni_dst):
    f32 = np.float32

    def deg(idx, n):
        return np.maximum(np.bincount(idx, minlength=n).astype(f32), 1.0)

    # normalized adjacency, dst rows in output-global order
    # (node 0:100 | pod 100:1600 | svc 1600:2000), src cols in xsrc order
    # (pod 0:1500 | node 1500:1600 | svc 1600:2000)
    A = np.zeros((2000, S), f32)
    si = deg(in_dst, N_NODE) ** -0.5
    so = deg(in_src, N_POD) ** -0.5
    np.add.at(A, (in_dst, in_src), si[in_dst] * so[in_src])
    si = deg(ni_dst, N_POD) ** -0.5
    so = deg(ni_src, N_NODE) ** -0.5
    np.add.at(A, (100 + ni_dst, 1500 + ni_src), si[ni_dst] * so[ni_src])
    si = deg(svc_dst, N_SVC) ** -0.5
    so = deg(svc_src, N_SVC) ** -0.5
    np.add.at(A, (1600 + svc_dst, 1600 + svc_src), si[svc_dst] * so[svc_src])

    xsrc = np.zeros((S, T * F), f32)
    xsrc[0:1500] = np.asarray(pod_feat, f32).reshape(N_POD, T * F)
    xsrc[1500:1600] = np.asarray(node_feat, f32).reshape(N_NODE, T * F)
    xsrc[1600:2000] = np.asarray(svc_feat, f32).reshape(N_SVC, T * F)
    xsrc = xsrc.astype(BF16)

    wconv = np.zeros((65, T, 384), f32)
    for ty, (W, b) in enumerate([(W_in, b_in), (W_ni, b_ni), (W_svc, b_svc)]):
        wconv[0:64, :, ty * 128:(ty + 1) * 128] = np.asarray(W, f32).transpose(1, 0, 2)
        wconv[64, :, ty * 128:(ty + 1) * 128] = np.asarray(b, f32)
    wconv = np.ascontiguousarray(wconv.reshape(65, T * 384)).astype(BF16)

    def wt(Wm, nk):
        Wt = _gate_reorder(Wm).T  # [K, 1024]
        return np.ascontiguousarray(
            Wt.reshape(nk, 128, 1024).transpose(1, 0, 2).reshape(128, nk * 1024)
        ).astype(BF16)

    def bvec(b_ih, b_hh):
        return _gate_reorder(
            (np.asarray(b_ih, f32) + np.asarray(b_hh, f32))[:, None])[:, 0]

    def brep(b):
        rep = np.broadcast_to(b.reshape(8, 128).T[:, :, None], (128, 8, B))
        return np.ascontiguousarray(rep.reshape(128, 8 * B)).astype(BF16)

    def bchunk(b):
        return np.ascontiguousarray(b.reshape(8, 128).T).astype(f32)

    bv0, bv1 = bvec(b_ih0, b_hh0), bvec(b_ih1, b_hh1)
    shared = dict(xsrc=xsrc, wconv=wconv,
                  wih0=wt(W_ih0, 1), whh0=wt(W_hh0, 2),
                  wih1=wt(W_ih1, 2), whh1=wt(W_hh1, 2),
                  b0rep=brep(bv0), b1rep=brep(bv1),
                  b0c=bchunk(bv0), b1c=bchunk(bv1),
                  brow=np.concatenate([bv0, bv1])[None, :].astype(BF16))
    in_maps = []
    for c in range(NCORES):
        nid, pid, sid = _core_row_ids(c)
        Ac = np.zeros((B, S), f32)
        Ac[NODE_OFF:NODE_OFF + len(nid)] = A[nid]
        Ac[POD_OFF:POD_OFF + len(pid)] = A[100 + pid]
        Ac[SVC_OFF:SVC_OFF + len(sid)] = A[1600 + sid]
        # pre-tiled to the SBUF layout [p, (k b)] so one contiguous DMA loads it
        at = np.ascontiguousarray(
            Ac.T.reshape(NK_SRC, 128, B).transpose(1, 0, 2).reshape(128, NK_SRC * B)
        ).astype(BF16)
        in_maps.append(dict(shared, at=at))
    return in_maps


def _assemble(results):
    full = np.empty((2000, T, H), np.float32)
    for c in range(NCORES):
        o = np.asarray(results[c]["out"], dtype=np.float32)  # [T,128,512]
        arr = o.reshape(T, 128, 2, 256).transpose(3, 0, 2, 1).reshape(256, T, H)
        nid, pid, sid = _core_row_ids(c)
        full[nid] = arr[NODE_OFF:NODE_OFF + len(nid)]
        full[100 + pid] = arr[POD_OFF:POD_OFF + len(pid)]
        full[1600 + sid] = arr[SVC_OFF:SVC_OFF + len(sid)]
    return full


def run(trace=False, tmpdir=None, **inputs):
    from concourse.bass_utils import run_bass_kernel_spmd
    nc = _get_built()
    in_maps = _prep_inputs(**inputs)
    res = run_bass_kernel_spmd(nc, in_maps, core_ids=list(range(NCORES)),
                               trace=trace, tmpdir=tmpdir)
    return _assemble(res.results), res


def kernel(**inputs):
    out, _ = run(trace=False, **inputs)
    return out


# revision 54
# speedup vs baseline: 1.0431x; 1.0431x over previous
"""Trainium2 Bass kernel for nn_AggrHGraphConvWindow (hetero GraphConv + 2-layer LSTM).

Sharding: data-parallel over the 2000-row batch axis across 8 NeuronCores.
Rows are padded/permuted to 2048 = 8 x 256 so every core runs an identical
program on blocks [node 13 | pod 188 | svc 50 | pad 5].  Source features are
replicated; each core holds the adjacency slice for its own destination rows,
so there are no collectives at all.

Device program per core (all matmuls bf16, PSUM fp32):
  conv stage 1:  agg_t^T [64,256] = sum_k xsrc_k[:, t]^T @ A^T_k
  conv stage 2:  x_t^T [128,256] = Wconv_ty^T @ [agg_t^T; ones-row]
                 (per-row-type weights; conv bias rides the ones row)
                 x_t = ACT Prelu(psum, alpha=.01) -> SBUF bf16  (leaky relu;
                 Prelu lives in the same ACT table set as sigmoid/tanh)
  LSTM (2 layers), gate blocks host-reordered to [i, g, f, o]:
      f,o matmuls emitted first (their sigmoid path feeds the cell update),
      then i,g; bias added on DVE from PSUM (bf16 out), sigmoid/tanh on ACT,
      cell update on DVE, tanh(c) + h-mul split in halves.
      The t-wavefront runs L1 TWO steps behind L0 so every L1 matmul operand
      is ready when its round starts and the L0 recurrence chain hides under
      L1's matmul work; conv timesteps are woven into the rounds to fill the
      remaining PE gaps.
      h2 DMA'd to DRAM [T,128,512] bf16; host un-permutes to [B,T,H] fp32.
"""

import numpy as np
import ml_dtypes

N_NODE, N_POD, N_SVC = 100, 1500, 400
T, F, IN, H = 32, 64, 128, 256
B = 256                       # rows per core (incl. padding)
NODE_OFF, POD_OFF, SVC_OFF = 0, 13, 201   # block offsets within a core's 256 rows
S = 2048                      # padded src rows: pod 0:1500 | node 1500:1600 | svc 1600:2000 | pad
NK_SRC = S // 128             # 16 k-tiles for conv stage 1
NCORES = 8
BF16 = ml_dtypes.bfloat16
GATE_PERM = [0, 2, 1, 3]      # new gate block order [i, g, f, o] (old i,f,g,o)
SIM_SAFE = False              # True: substitute Relu for Lrelu (CoreSim lacks Lrelu)

_BUILT = None   # cached compiled Bass program


def _build():
    import concourse.bass as bass  # noqa: F401
    import concourse.mybir as mybir
    import concourse.tile as tile
    from concourse import bacc
    from contextlib import ExitStack

    f32 = mybir.dt.float32
    bf16 = mybir.dt.bfloat16
    AF = mybir.ActivationFunctionType

    nc = bacc.Bacc("TRN2", target_bir_lowering=False, debug=False,
                   num_devices=NCORES)

    xsrc_d = nc.dram_tensor("xsrc", [S, T * F], bf16, kind="ExternalInput")
    at_d = nc.dram_tensor("at", [128, NK_SRC * B], bf16, kind="ExternalInput")
    wconv_d = nc.dram_tensor("wconv", [65, T * 384], bf16, kind="ExternalInput")
    wih0_d = nc.dram_tensor("wih0", [128, 1024], bf16, kind="ExternalInput")
    whh0_d = nc.dram_tensor("whh0", [128, 2048], bf16, kind="ExternalInput")
    wih1_d = nc.dram_tensor("wih1", [128, 2048], bf16, kind="ExternalInput")
    whh1_d = nc.dram_tensor("whh1", [128, 2048], bf16, kind="ExternalInput")
    b0_d = nc.dram_tensor("b0rep", [128, 2048], bf16, kind="ExternalInput")
    b1_d = nc.dram_tensor("b1rep", [128, 2048], bf16, kind="ExternalInput")
    b0c_d = nc.dram_tensor("b0c", [128, 8], f32, kind="ExternalInput")
    b1c_d = nc.dram_tensor("b1c", [128, 8], f32, kind="ExternalInput")
    brow_d = nc.dram_tensor("brow", [1, 2048], bf16, kind="ExternalInput")
    out_d = nc.dram_tensor("out", [T, 128, 512], bf16, kind="ExternalOutput")

    with tile.TileContext(nc) as tc, ExitStack() as ctx:
        const = ctx.enter_context(tc.tile_pool(name="const", bufs=1))
        psp = ctx.enter_context(tc.tile_pool(name="psp", bufs=4, space="PSUM"))
        gp = ctx.enter_context(tc.tile_pool(name="gp", bufs=4))
        wk = ctx.enter_context(tc.tile_pool(name="wk", bufs=2))
        aggp = ctx.enter_context(tc.tile_pool(name="aggp", bufs=4))
        wcp = ctx.enter_context(tc.tile_pool(name="wcp", bufs=3))
        hop = ctx.enter_context(tc.tile_pool(name="hop", bufs=2))
        cp = ctx.enter_context(tc.tile_pool(name="cp", bufs=4))

        # ---- persistent loads ----
        # small operands first so conv matmuls can start as xsrc k-tiles land
        at = const.tile([128, NK_SRC * B], bf16)
        nc.sync.dma_start(out=at[:], in_=at_d.ap())
        wih0 = const.tile([128, 1024], bf16)
        nc.sync.dma_start(out=wih0[:], in_=wih0_d.ap())
        xsrc = const.tile([128, NK_SRC * 2048], bf16)
        for k in range(NK_SRC):
            nc.sync.dma_start(out=xsrc[:, k * 2048:(k + 1) * 2048],
                              in_=xsrc_d.ap()[k * 128:(k + 1) * 128, :])
        whh0 = const.tile([128, 2048], bf16)
        nc.sync.dma_start(out=whh0[:], in_=whh0_d.ap())
        wih1 = const.tile([128, 2048], bf16)
        nc.sync.dma_start(out=wih1[:], in_=wih1_d.ap())
        whh1 = const.tile([128, 2048], bf16)
        nc.sync.dma_start(out=whh1[:], in_=whh1_d.ap())
        b0rep = const.tile([128, 2048], bf16)
        nc.sync.dma_start(out=b0rep[:], in_=b0_d.ap())
        b1rep = const.tile([128, 2048], bf16)
        nc.sync.dma_start(out=b1rep[:], in_=b1_d.ap())
        b0c = const.tile([128, 8], f32)
        nc.sync.dma_start(out=b0c[:], in_=b0c_d.ap())
        b1c = const.tile([128, 8], f32)
        nc.sync.dma_start(out=b1c[:], in_=b1c_d.ap())
        brow = const.tile([1, 2048], bf16)
        nc.sync.dma_start(out=brow[:], in_=brow_d.ap())
        ones = const.tile([1, B], bf16)
        nc.vector.memset(ones[:], 1.0)

        x_sb = const.tile([128, T * B], bf16)        # conv output (LSTM L0 input)
        h1f = const.tile([128, T * 512], bf16)       # L0 hidden states (L1 input)

        # ---- conv ----
        blocks = [(NODE_OFF, POD_OFF - NODE_OFF),
                  (POD_OFF, SVC_OFF - POD_OFF),
                  (SVC_OFF, B - SVC_OFF)]
        def conv_t(t):
            agg = psp.tile([64, B], f32, tag="psA", bufs=4)
            for k in range(NK_SRC):
                nc.tensor.matmul(
                    agg[:, :],
                    xsrc[:, k * 2048 + t * 64: k * 2048 + (t + 1) * 64],
                    at[:, k * B:(k + 1) * B],
                    start=(k == 0), stop=(k == NK_SRC - 1))
            aggT = aggp.tile([65, B], bf16, tag="aggT")
            nc.vector.tensor_copy(aggT[0:64, :], agg[:, :])
            nc.gpsimd.memset(aggT[64:65, :], 1.0)
            wct = wcp.tile([65, 384], bf16, tag="wct")
            nc.sync.dma_start(out=wct[:],
                              in_=wconv_d.ap()[:, t * 384:(t + 1) * 384])
            xps = psp.tile([128, B], f32, tag="psA", bufs=4)
            for ty, (off, wid) in enumerate(blocks):
                nc.tensor.matmul(xps[:, off:off + wid],
                                 wct[:, ty * 128:(ty + 1) * 128],
                                 aggT[:, off:off + wid],
                                 start=True, stop=True)
            nc.scalar.activation(x_sb[:, t * B:(t + 1) * B], xps[:, :],
                                 AF.Relu if SIM_SAFE else AF.Prelu,
                                 alpha=0.01)

        # ---- LSTM ----
        state = {0: {"c": None}, 1: {"c": None, "h": None}}

        def lstm_step(layer, t):
            # kparts ordered so ready-early operands issue first
            if layer == 0:
                kparts = [(wih0, 1, x_sb[:, t * B:(t + 1) * B])]
                if t > 0:
                    kparts.append((whh0, 2, h1f[:, (t - 1) * 512: t * 512]))
                brep, bc, boff = b0rep, b0c, 0
            else:
                kparts = [(whh1, 2, state[1]["h"][:])] if t > 0 else []
                kparts.append((wih1, 2, h1f[:, t * 512:(t + 1) * 512]))
                brep, bc, boff = b1rep, b1c, 1024
            nmm = sum(nk for (_, nk, _) in kparts)

            def emit_mms(ps, pcol, cth):
                i_mm = 0
                for (w, nk, rhs) in kparts:
                    for kk in range(nk):
                        nc.tensor.matmul(
                            ps[:, pcol * 256:(pcol + 1) * 256],
                            w[:, kk * 1024 + cth * 128: kk * 1024 + (cth + 1) * 128],
                            rhs[:, kk * B:(kk + 1) * B],
                            start=(i_mm == 0), stop=(i_mm == nmm - 1))
                        i_mm += 1

            # gate order [i, g, f, o]: i,g chain-critical -> per-chunk biased
            # ACT straight from PSUM; f,o -> wide DVE bias-add + sigmoid.
            # f,o matmuls first: their sigmoid path (p2 = f*c) must be ready
            # by the time the i,g chain reaches the cell update
            psB = psp.tile([128, 1024], f32, tag="psB", bufs=2)
            for c in range(4):
                emit_mms(psB, c, 4 + c)        # chunks 4-7 = gates f,o
            psi = psp.tile([128, 512], f32, tag="psA", bufs=4)
            for c in range(2):
                emit_mms(psi, c, c)            # chunks 0,1 = gate i
            psg = psp.tile([128, 512], f32, tag="psA", bufs=4)
            for c in range(2):
                emit_mms(psg, c, 2 + c)        # chunks 2,3 = gate g
            # f half first: p2 = f*c must be ready before the cell add
            zf = wk.tile([128, 512], bf16, tag="zf")
            nc.vector.tensor_add(zf[:], psB[:, 0:512], brep[:, 1024:1536])
            gfT = gp.tile([128, 512], bf16, tag="gfT")
            nc.scalar.activation(gfT[:], zf[:], AF.Sigmoid)
            zo = wk.tile([128, 512], bf16, tag="zo")
            nc.vector.tensor_add(zo[:], psB[:, 512:1024], brep[:, 1536:2048])
            goT = gp.tile([128, 512], bf16, tag="goT")
            nc.scalar.activation(goT[:], zo[:], AF.Sigmoid)
            gf, go = gfT[:], goT[:]
            zi = wk.tile([128, 512], bf16, tag="zi")
            nc.vector.tensor_add(zi[:], psi[:], brep[:, 0:512])
            gi = gp.tile([128, 512], bf16, tag="gi")
            nc.scalar.activation(gi[:], zi[:], AF.Sigmoid)
            zg = wk.tile([128, 512], bf16, tag="zg")
            nc.vector.tensor_add(zg[:], psg[:], brep[:, 512:1024])
            gg = gp.tile([128, 512], bf16, tag="gg")
            nc.scalar.activation(gg[:], zg[:], AF.Tanh)

            p1 = wk.tile([128, 512], bf16, tag="p1")
            nc.vector.tensor_mul(p1[:], gi[:], gg[:])
            cn = cp.tile([128, 512], bf16, tag=f"c{layer}")
            if t == 0:
                nc.vector.tensor_copy(cn[:], p1[:])
            else:
                p2 = wk.tile([128, 512], bf16, tag="p2")
                nc.vector.tensor_mul(p2[:], gf, state[layer]["c"][:])
                nc.vector.tensor_add(cn[:], p1[:], p2[:])
            state[layer]["c"] = cn
            thc = wk.tile([128, 512], bf16, tag="thc")
            if layer == 0:
                hout = h1f[:, t * 512:(t + 1) * 512]
            else:
                hout = hop.tile([128, 512], bf16, tag="h2o")
                state[1]["h"] = hout
            # halves: downstream matmuls consume h k-tile by k-tile
            for hh in range(2):
                sl = slice(hh * 256, (hh + 1) * 256)
                nc.scalar.activation(thc[:, sl], cn[:, sl], AF.Tanh)
                nc.vector.tensor_mul(hout[:, sl],
                                     goT[:, hh * 256:(hh + 1) * 256],
                                     thc[:, sl])
            if layer == 1:
                nc.sync.dma_start(out=out_d.ap()[t], in_=hout[:])

        # weave conv pairs into the LSTM wavefront; L1 lags L0 by TWO
        # steps so every L1 matmul input is ready before the round starts
        # and the L0 recurrence chain hides under L1's matmul work
        conv_t(0)
        conv_t(1)
        conv_t(2)
        lstm_step(0, 0)
        lstm_step(0, 1)
        for t in range(2, T):
            lstm_step(1, t - 2)
            lstm_step(0, t)
            if t + 1 < T:
                conv_t(t + 1)
        lstm_step(1, T - 2)
        lstm_step(1, T - 1)

    nc.compile()
    return nc


def _get_built():
    global _BUILT
    if _BUILT is None:
        _BUILT = _build()
    return _BUILT


def _core_row_ids(c):
    """Per-core (node_ids, pod_ids, svc_ids) real row id arrays."""
    n0, n1 = 13 * c, min(13 * (c + 1), N_NODE)
    p0, p1 = 188 * c, min(188 * (c + 1), N_POD)
    s0, s1 = 50 * c, min(50 * (c + 1), N_SVC)
    return np.arange(n0, n1), np.arange(p0, p1), np.arange(s0, s1)


def _gate_reorder(Wm):
    Wm = np.asarray(Wm, np.float32)
    return np.concatenate([Wm[g * 256:(g + 1) * 256] for g in GATE_PERM], axis=0)


def _prep_inputs(node_feat, pod_feat, svc_feat, W_svc, b_svc, W_in, b_in,
                 W_ni, b_ni, W_ih0, W_hh0, b_ih0, b_hh0, W_ih1, W_hh1,
                 b_ih1, b_hh1, svc_src, svc_dst, in_src, in_dst,
                 ni_src, ni_dst):
    f32 = np.float32

    def deg(idx, n):
        return np.maximum(np.bincount(idx, minlength=n).astype(f32), 1.0)

    # normalized adjacency, dst rows in output-global order
    # (node 0:100 | pod 100:1600 | svc 1600:2000), src cols in xsrc order
    # (pod 0:1500 | node 1500:1600 | svc 1600:2000)
    A = np.zeros((2000, S), f32)
    si = deg(in_dst, N_NODE) ** -0.5
    so = deg(in_src, N_POD) ** -0.5
    np.add.at(A, (in_dst, in_src), si[in_dst] * so[in_src])
    si = deg(ni_dst, N_POD) ** -0.5
    so = deg(ni_src, N_NODE) ** -0.5
    np.add.at(A, (100 + ni_dst, 1500 + ni_src), si[ni_dst] * so[ni_src])
    si = deg(svc_dst, N_SVC) ** -0.5
    so = deg(svc_src, N_SVC) ** -0.5
    np.add.at(A, (1600 + svc_dst, 1600 + svc_src), si[svc_dst] * so[svc_src])

    xsrc = np.zeros((S, T * F), f32)
    xsrc[0:1500] = np.asarray(pod_feat, f32).reshape(N_POD, T * F)
    xsrc[1500:1600] = np.asarray(node_feat, f32).reshape(N_NODE, T * F)
    xsrc[1600:2000] = np.asarray(svc_feat, f32).reshape(N_SVC, T * F)
    xsrc = xsrc.astype(BF16)

    wconv = np.zeros((65, T, 384), f32)
    for ty, (W, b) in enumerate([(W_in, b_in), (W_ni, b_ni), (W_svc, b_svc)]):
        wconv[0:64, :, ty * 128:(ty + 1) * 128] = np.asarray(W, f32).transpose(1, 0, 2)
        wconv[64, :, ty * 128:(ty + 1) * 128] = np.asarray(b, f32)
    wconv = np.ascontiguousarray(wconv.reshape(65, T * 384)).astype(BF16)

    def wt(Wm, nk):
        Wt = _gate_reorder(Wm).T  # [K, 1024]
        return np.ascontiguousarray(
            Wt.reshape(nk, 128, 1024).transpose(1, 0, 2).reshape(128, nk * 1024)
        ).astype(BF16)

    def bvec(b_ih, b_hh):
        return _gate_reorder(
            (np.asarray(b_ih, f32) + np.asarray(b_hh, f32))[:, None])[:, 0]

    def brep(b):
        rep = np.broadcast_to(b.reshape(8, 128).T[:, :, None], (128, 8, B))
        return np.ascontiguousarray(rep.reshape(128, 8 * B)).astype(BF16)

    def bchunk(b):
        return np.ascontiguousarray(b.reshape(8, 128).T).astype(f32)

    bv0, bv1 = bvec(b_ih0, b_hh0), bvec(b_ih1, b_hh1)
    shared = dict(xsrc=xsrc, wconv=wconv,
                  wih0=wt(W_ih0, 1), whh0=wt(W_hh0, 2),
                  wih1=wt(W_ih1, 2), whh1=wt(W_hh1, 2),
                  b0rep=brep(bv0), b1rep=brep(bv1),
                  b0c=bchunk(bv0), b1c=bchunk(bv1),
                  brow=np.concatenate([bv0, bv1])[None, :].astype(BF16))
    in_maps = []
    for c in range(NCORES):
        nid, pid, sid = _core_row_ids(c)
        Ac = np.zeros((B, S), f32)
        Ac[NODE_OFF:NODE_OFF + len(nid)] = A[nid]
        Ac[POD_OFF:POD_OFF + len(pid)] = A[100 + pid]
        Ac[SVC_OFF:SVC_OFF + len(sid)] = A[1600 + sid]
        # pre-tiled to the SBUF layout [p, (k b)] so one contiguous DMA loads it
        at = np.ascontiguousarray(
            Ac.T.reshape(NK_SRC, 128, B).transpose(1, 0, 2).reshape(128, NK_SRC * B)
        ).astype(BF16)
        in_maps.append(dict(shared, at=at))
    return in_maps


def _assemble(results):
    full = np.empty((2000, T, H), np.float32)
    for c in range(NCORES):
        o = np.asarray(results[c]["out"], dtype=np.float32)  # [T,128,512]
        arr = o.reshape(T, 128, 2, 256).transpose(3, 0, 2, 1).reshape(256, T, H)
        nid, pid, sid = _core_row_ids(c)
        full[nid] = arr[NODE_OFF:NODE_OFF + len(nid)]
        full[100 + pid] = arr[POD_OFF:POD_OFF + len(pid)]
        full[1600 + sid] = arr[SVC_OFF:SVC_OFF + len(sid)]
    return full


def run(trace=False, tmpdir=None, **inputs):
    from concourse.bass_utils import run_bass_kernel_spmd
    nc = _get_built()
    in_maps = _prep_inputs(**inputs)
    res = run_bass_kernel_spmd(nc, in_maps, core_ids=list(range(NCORES)),
                               trace=trace, tmpdir=tmpdir)
    return _assemble(res.results), res


def kernel(**inputs):
    out, _ = run(trace=False, **inputs)
    return out


# revision 55
# speedup vs baseline: 1.0486x; 1.0053x over previous
"""Trainium2 Bass kernel for nn_AggrHGraphConvWindow (hetero GraphConv + 2-layer LSTM).

Sharding: data-parallel over the 2000-row batch axis across 8 NeuronCores.
Rows are padded/permuted to 2048 = 8 x 256 so every core runs an identical
program on blocks [node 13 | pod 188 | svc 50 | pad 5].  Source features are
replicated; each core holds the adjacency slice for its own destination rows,
so there are no collectives at all.

Device program per core (all matmuls bf16, PSUM fp32):
  conv stage 1:  agg_t^T [64,256] = sum_k xsrc_k[:, t]^T @ A^T_k
  conv stage 2:  x_t^T [128,256] = Wconv_ty^T @ [agg_t^T; ones-row]
                 (per-row-type weights; conv bias rides the ones row)
                 x_t = ACT Prelu(psum, alpha=.01) -> SBUF bf16  (leaky relu;
                 Prelu lives in the same ACT table set as sigmoid/tanh)
  LSTM (2 layers), gate blocks host-reordered to [i, g, f, o]:
      f,o matmuls emitted first (their sigmoid path feeds the cell update),
      then i,g; bias added on DVE from PSUM (bf16 out), sigmoid/tanh on ACT,
      cell update on DVE, tanh(c) + h-mul split in halves.
      The t-wavefront runs L1 TWO steps behind L0 so every L1 matmul operand
      is ready when its round starts and the L0 recurrence chain hides under
      L1's matmul work; conv timesteps are woven into the rounds to fill the
      remaining PE gaps.
      h2 DMA'd to DRAM [T,128,512] bf16; host un-permutes to [B,T,H] fp32.
"""

import numpy as np
import ml_dtypes

N_NODE, N_POD, N_SVC = 100, 1500, 400
T, F, IN, H = 32, 64, 128, 256
B = 256                       # rows per core (incl. padding)
NODE_OFF, POD_OFF, SVC_OFF = 0, 13, 201   # block offsets within a core's 256 rows
S = 2048                      # padded src rows: pod 0:1500 | node 1500:1600 | svc 1600:2000 | pad
NK_SRC = S // 128             # 16 k-tiles for conv stage 1
NCORES = 8
BF16 = ml_dtypes.bfloat16
GATE_PERM = [0, 2, 1, 3]      # new gate block order [i, g, f, o] (old i,f,g,o)
SIM_SAFE = False              # True: substitute Relu for Lrelu (CoreSim lacks Lrelu)

_BUILT = None   # cached compiled Bass program


def _build():
    import concourse.bass as bass  # noqa: F401
    import concourse.mybir as mybir
    import concourse.tile as tile
    from concourse import bacc
    from contextlib import ExitStack

    f32 = mybir.dt.float32
    bf16 = mybir.dt.bfloat16
    AF = mybir.ActivationFunctionType

    nc = bacc.Bacc("TRN2", target_bir_lowering=False, debug=False,
                   num_devices=NCORES)

    xsrc_d = nc.dram_tensor("xsrc", [S, T * F], bf16, kind="ExternalInput")
    at_d = nc.dram_tensor("at", [128, NK_SRC * B], bf16, kind="ExternalInput")
    wconv_d = nc.dram_tensor("wconv", [65, T * 384], bf16, kind="ExternalInput")
    wih0_d = nc.dram_tensor("wih0", [128, 1024], bf16, kind="ExternalInput")
    whh0_d = nc.dram_tensor("whh0", [128, 2048], bf16, kind="ExternalInput")
    wih1_d = nc.dram_tensor("wih1", [128, 2048], bf16, kind="ExternalInput")
    whh1_d = nc.dram_tensor("whh1", [128, 2048], bf16, kind="ExternalInput")
    b0_d = nc.dram_tensor("b0rep", [128, 2048], bf16, kind="ExternalInput")
    b1_d = nc.dram_tensor("b1rep", [128, 2048], bf16, kind="ExternalInput")
    b0c_d = nc.dram_tensor("b0c", [128, 8], f32, kind="ExternalInput")
    b1c_d = nc.dram_tensor("b1c", [128, 8], f32, kind="ExternalInput")
    brow_d = nc.dram_tensor("brow", [1, 2048], bf16, kind="ExternalInput")
    out_d = nc.dram_tensor("out", [T, 128, 512], bf16, kind="ExternalOutput")

    with tile.TileContext(nc) as tc, ExitStack() as ctx:
        const = ctx.enter_context(tc.tile_pool(name="const", bufs=1))
        psp = ctx.enter_context(tc.tile_pool(name="psp", bufs=4, space="PSUM"))
        gp = ctx.enter_context(tc.tile_pool(name="gp", bufs=4))
        wk = ctx.enter_context(tc.tile_pool(name="wk", bufs=2))
        aggp = ctx.enter_context(tc.tile_pool(name="aggp", bufs=4))
        wcp = ctx.enter_context(tc.tile_pool(name="wcp", bufs=3))
        hop = ctx.enter_context(tc.tile_pool(name="hop", bufs=2))
        cp = ctx.enter_context(tc.tile_pool(name="cp", bufs=4))

        # ---- persistent loads ----
        # small operands first so conv matmuls can start as xsrc k-tiles land
        at = const.tile([128, NK_SRC * B], bf16)
        nc.sync.dma_start(out=at[:], in_=at_d.ap())
        wih0 = const.tile([128, 1024], bf16)
        nc.sync.dma_start(out=wih0[:], in_=wih0_d.ap())
        xsrc = const.tile([128, NK_SRC * 2048], bf16)
        for k in range(NK_SRC):
            nc.sync.dma_start(out=xsrc[:, k * 2048:(k + 1) * 2048],
                              in_=xsrc_d.ap()[k * 128:(k + 1) * 128, :])
        whh0 = const.tile([128, 2048], bf16)
        nc.sync.dma_start(out=whh0[:], in_=whh0_d.ap())
        wih1 = const.tile([128, 2048], bf16)
        nc.sync.dma_start(out=wih1[:], in_=wih1_d.ap())
        whh1 = const.tile([128, 2048], bf16)
        nc.sync.dma_start(out=whh1[:], in_=whh1_d.ap())
        b0rep = const.tile([128, 2048], bf16)
        nc.sync.dma_start(out=b0rep[:], in_=b0_d.ap())
        b1rep = const.tile([128, 2048], bf16)
        nc.sync.dma_start(out=b1rep[:], in_=b1_d.ap())
        b0c = const.tile([128, 8], f32)
        nc.sync.dma_start(out=b0c[:], in_=b0c_d.ap())
        b1c = const.tile([128, 8], f32)
        nc.sync.dma_start(out=b1c[:], in_=b1c_d.ap())
        brow = const.tile([1, 2048], bf16)
        nc.sync.dma_start(out=brow[:], in_=brow_d.ap())
        ones = const.tile([1, B], bf16)
        nc.vector.memset(ones[:], 1.0)

        x_sb = const.tile([128, T * B], bf16)        # conv output (LSTM L0 input)
        h1f = const.tile([128, T * 512], bf16)       # L0 hidden states (L1 input)

        # ---- conv ----
        blocks = [(NODE_OFF, POD_OFF - NODE_OFF),
                  (POD_OFF, SVC_OFF - POD_OFF),
                  (SVC_OFF, B - SVC_OFF)]
        def conv_t(t):
            agg = psp.tile([64, B], f32, tag="psA", bufs=4)
            for k in range(NK_SRC):
                nc.tensor.matmul(
                    agg[:, :],
                    xsrc[:, k * 2048 + t * 64: k * 2048 + (t + 1) * 64],
                    at[:, k * B:(k + 1) * B],
                    start=(k == 0), stop=(k == NK_SRC - 1))
            aggT = aggp.tile([65, B], bf16, tag="aggT")
            nc.vector.tensor_copy(aggT[0:64, :], agg[:, :])
            nc.gpsimd.memset(aggT[64:65, :], 1.0)
            wct = wcp.tile([65, 384], bf16, tag="wct")
            nc.sync.dma_start(out=wct[:],
                              in_=wconv_d.ap()[:, t * 384:(t + 1) * 384])
            xps = psp.tile([128, B], f32, tag="psA", bufs=4)
            for ty, (off, wid) in enumerate(blocks):
                nc.tensor.matmul(xps[:, off:off + wid],
                                 wct[:, ty * 128:(ty + 1) * 128],
                                 aggT[:, off:off + wid],
                                 start=True, stop=True)
            nc.scalar.activation(x_sb[:, t * B:(t + 1) * B], xps[:, :],
                                 AF.Relu if SIM_SAFE else AF.Prelu,
                                 alpha=0.01)

        # ---- LSTM ----
        state = {0: {"c": None}, 1: {"c": None, "h": None}}

        def lstm_step(layer, t):
            # kparts ordered so ready-early operands issue first
            if layer == 0:
                kparts = [(wih0, 1, x_sb[:, t * B:(t + 1) * B])]
                if t > 0:
                    kparts.append((whh0, 2, h1f[:, (t - 1) * 512: t * 512]))
                brep, bc, boff = b0rep, b0c, 0
            else:
                kparts = [(whh1, 2, state[1]["h"][:])] if t > 0 else []
                kparts.append((wih1, 2, h1f[:, t * 512:(t + 1) * 512]))
                brep, bc, boff = b1rep, b1c, 1024
            nmm = sum(nk for (_, nk, _) in kparts)

            def emit_mms(ps, pcol, cth):
                i_mm = 0
                for (w, nk, rhs) in kparts:
                    for kk in range(nk):
                        nc.tensor.matmul(
                            ps[:, pcol * 256:(pcol + 1) * 256],
                            w[:, kk * 1024 + cth * 128: kk * 1024 + (cth + 1) * 128],
                            rhs[:, kk * B:(kk + 1) * B],
                            start=(i_mm == 0), stop=(i_mm == nmm - 1))
                        i_mm += 1

            # gate order [i, g, f, o]: i,g chain-critical -> per-chunk biased
            # ACT straight from PSUM; f,o -> wide DVE bias-add + sigmoid.
            # f,o matmuls first: their sigmoid path (p2 = f*c) must be ready
            # by the time the i,g chain reaches the cell update
            psB = psp.tile([128, 1024], f32, tag="psB", bufs=2)
            for c in range(4):
                emit_mms(psB, c, 4 + c)        # chunks 4-7 = gates f,o
            psi = psp.tile([128, 512], f32, tag="psA", bufs=4)
            for c in range(2):
                emit_mms(psi, c, c)            # chunks 0,1 = gate i
            psg = psp.tile([128, 512], f32, tag="psA", bufs=4)
            for c in range(2):
                emit_mms(psg, c, 2 + c)        # chunks 2,3 = gate g
            # f half first: p2 = f*c must be ready before the cell add
            zf = wk.tile([128, 512], bf16, tag="zf")
            nc.vector.tensor_add(zf[:], psB[:, 0:512], brep[:, 1024:1536])
            gfT = gp.tile([128, 512], bf16, tag="gfT")
            nc.scalar.activation(gfT[:], zf[:], AF.Sigmoid)
            zo = wk.tile([128, 512], bf16, tag="zo")
            nc.vector.tensor_add(zo[:], psB[:, 512:1024], brep[:, 1536:2048])
            goT = gp.tile([128, 512], bf16, tag="goT")
            nc.scalar.activation(goT[:], zo[:], AF.Sigmoid)
            gf, go = gfT[:], goT[:]
            zi = wk.tile([128, 512], bf16, tag="zi")
            nc.vector.tensor_add(zi[:], psi[:], brep[:, 0:512])
            gi = gp.tile([128, 512], bf16, tag="gi")
            nc.scalar.activation(gi[:], zi[:], AF.Sigmoid)
            zg = wk.tile([128, 512], bf16, tag="zg")
            nc.vector.tensor_add(zg[:], psg[:], brep[:, 512:1024])
            gg = gp.tile([128, 512], bf16, tag="gg")
            nc.scalar.activation(gg[:], zg[:], AF.Tanh)

            cn = cp.tile([128, 512], bf16, tag=f"c{layer}")
            if t == 0:
                nc.vector.tensor_mul(cn[:], gi[:], gg[:])
            else:
                # p2 first: its input (sigma(f), c) is ready before p1's
                p2 = wk.tile([128, 512], bf16, tag="p2")
                nc.vector.tensor_mul(p2[:], gf, state[layer]["c"][:])
                p1 = wk.tile([128, 512], bf16, tag="p1")
                nc.vector.tensor_mul(p1[:], gi[:], gg[:])
                nc.vector.tensor_add(cn[:], p1[:], p2[:])
            state[layer]["c"] = cn
            thc = wk.tile([128, 512], bf16, tag="thc")
            if layer == 0:
                hout = h1f[:, t * 512:(t + 1) * 512]
            else:
                hout = hop.tile([128, 512], bf16, tag="h2o")
                state[1]["h"] = hout
            # halves: downstream matmuls consume h k-tile by k-tile
            for hh in range(2):
                sl = slice(hh * 256, (hh + 1) * 256)
                nc.scalar.activation(thc[:, sl], cn[:, sl], AF.Tanh)
                nc.vector.tensor_mul(hout[:, sl],
                                     goT[:, hh * 256:(hh + 1) * 256],
                                     thc[:, sl])
            if layer == 1:
                nc.sync.dma_start(out=out_d.ap()[t], in_=hout[:])

        # weave conv pairs into the LSTM wavefront; L1 lags L0 by TWO
        # steps so every L1 matmul input is ready before the round starts
        # and the L0 recurrence chain hides under L1's matmul work
        conv_t(0)
        conv_t(1)
        conv_t(2)
        lstm_step(0, 0)
        lstm_step(0, 1)
        for t in range(2, T):
            lstm_step(1, t - 2)
            lstm_step(0, t)
            if t + 1 < T:
                conv_t(t + 1)
        lstm_step(1, T - 2)
        lstm_step(1, T - 1)

    nc.compile()
    return nc


def _get_built():
    global _BUILT
    if _BUILT is None:
        _BUILT = _build()
    return _BUILT


def _core_row_ids(c):
    """Per-core (node_ids, pod_ids, svc_ids) real row id arrays."""
    n0, n1 = 13 * c, min(13 * (c + 1), N_NODE)
    p0, p1 = 188 * c, min(188 * (c + 1), N_POD)
    s0, s1 = 50 * c, min(50 * (c + 1), N_SVC)
    return np.arange(n0, n1), np.arange(p0, p1), np.arange(s0, s1)


def _gate_reorder(Wm):
    Wm = np.asarray(Wm, np.float32)
    return np.concatenate([Wm[g * 256:(g + 1) * 256] for g in GATE_PERM], axis=0)


def _prep_inputs(node_feat, pod_feat, svc_feat, W_svc, b_svc, W_in, b_in,
                 W_ni, b_ni, W_ih0, W_hh0, b_ih0, b_hh0, W_ih1, W_hh1,
                 b_ih1, b_hh1, svc_src, svc_dst, in_src, in_dst,
                 ni_src, ni_dst):
    f32 = np.float32

    def deg(idx, n):
        return np.maximum(np.bincount(idx, minlength=n).astype(f32), 1.0)

    # normalized adjacency, dst rows in output-global order
    # (node 0:100 | pod 100:1600 | svc 1600:2000), src cols in xsrc order
    # (pod 0:1500 | node 1500:1600 | svc 1600:2000)
    A = np.zeros((2000, S), f32)
    si = deg(in_dst, N_NODE) ** -0.5
    so = deg(in_src, N_POD) ** -0.5
    np.add.at(A, (in_dst, in_src), si[in_dst] * so[in_src])
    si = deg(ni_dst, N_POD) ** -0.5
    so = deg(ni_src, N_NODE) ** -0.5
    np.add.at(A, (100 + ni_dst, 1500 + ni_src), si[ni_dst] * so[ni_src])
    si = deg(svc_dst, N_SVC) ** -0.5
    so = deg(svc_src, N_SVC) ** -0.5
    np.add.at(A, (1600 + svc_dst, 1600 + svc_src), si[svc_dst] * so[svc_src])

    xsrc = np.zeros((S, T * F), f32)
    xsrc[0:1500] = np.asarray(pod_feat, f32).reshape(N_POD, T * F)
    xsrc[1500:1600] = np.asarray(node_feat, f32).reshape(N_NODE, T * F)
    xsrc[1600:2000] = np.asarray(svc_feat, f32).reshape(N_SVC, T * F)
    xsrc = xsrc.astype(BF16)

    wconv = np.zeros((65, T, 384), f32)
    for ty, (W, b) in enumerate([(W_in, b_in), (W_ni, b_ni), (W_svc, b_svc)]):
        wconv[0:64, :, ty * 128:(ty + 1) * 128] = np.asarray(W, f32).transpose(1, 0, 2)
        wconv[64, :, ty * 128:(ty + 1) * 128] = np.asarray(b, f32)
    wconv = np.ascontiguousarray(wconv.reshape(65, T * 384)).astype(BF16)

    def wt(Wm, nk):
        Wt = _gate_reorder(Wm).T  # [K, 1024]
        return np.ascontiguousarray(
            Wt.reshape(nk, 128, 1024).transpose(1, 0, 2).reshape(128, nk * 1024)
        ).astype(BF16)

    def bvec(b_ih, b_hh):
        return _gate_reorder(
            (np.asarray(b_ih, f32) + np.asarray(b_hh, f32))[:, None])[:, 0]

    def brep(b):
        rep = np.broadcast_to(b.reshape(8, 128).T[:, :, None], (128, 8, B))
        return np.ascontiguousarray(rep.reshape(128, 8 * B)).astype(BF16)

    def bchunk(b):
        return np.ascontiguousarray(b.reshape(8, 128).T).astype(f32)

    bv0, bv1 = bvec(b_ih0, b_hh0), bvec(b_ih1, b_hh1)
    shared = dict(xsrc=xsrc, wconv=wconv,
                  wih0=wt(W_ih0, 1), whh0=wt(W_hh0, 2),
                  wih1=wt(W_ih1, 2), whh1=wt(W_hh1, 2),
                  b0rep=brep(bv0), b1rep=brep(bv1),
                  b0c=bchunk(bv0), b1c=bchunk(bv1),
                  brow=np.concatenate([bv0, bv1])[None, :].astype(BF16))
    in_maps = []
    for c in range(NCORES):
        nid, pid, sid = _core_row_ids(c)
        Ac = np.zeros((B, S), f32)
        Ac[NODE_OFF:NODE_OFF + len(nid)] = A[nid]
        Ac[POD_OFF:POD_OFF + len(pid)] = A[100 + pid]
        Ac[SVC_OFF:SVC_OFF + len(sid)] = A[1600 + sid]
        # pre-tiled to the SBUF layout [p, (k b)] so one contiguous DMA loads it
        at = np.ascontiguousarray(
            Ac.T.reshape(NK_SRC, 128, B).transpose(1, 0, 2).reshape(128, NK_SRC * B)
        ).astype(BF16)
        in_maps.append(dict(shared, at=at))
    return in_maps


def _assemble(results):
    full = np.empty((2000, T, H), np.float32)
    for c in range(NCORES):
        o = np.asarray(results[c]["out"], dtype=np.float32)  # [T,128,512]
        arr = o.reshape(T, 128, 2, 256).transpose(3, 0, 2, 1).reshape(256, T, H)
        nid, pid, sid = _core_row_ids(c)
        full[nid] = arr[NODE_OFF:NODE_OFF + len(nid)]
        full[100 + pid] = arr[POD_OFF:POD_OFF + len(pid)]
        full[1600 + sid] = arr[SVC_OFF:SVC_OFF + len(sid)]
    return full


def run(trace=False, tmpdir=None, **inputs):
    from concourse.bass_utils import run_bass_kernel_spmd
    nc = _get_built()
    in_maps = _prep_inputs(**inputs)
    res = run_bass_kernel_spmd(nc, in_maps, core_ids=list(range(NCORES)),
                               trace=trace, tmpdir=tmpdir)
    return _assemble(res.results), res


def kernel(**inputs):
    out, _ = run(trace=False, **inputs)
    return out
